# revision 15
# baseline (speedup 1.0000x reference)
"""Trainium2 Bass kernel for nn_Attention_6201932775733 (sparse window attention).

Strategy (8 NeuronCores, SPMD, data-parallel over (batch, row-stripe)):
  - Core i handles batch i//4, two 16-row stripes {2*(i%4), 2*(i%4)+1}.
  - Inputs host-permuted to window-major position order, packed [128, kt, pos]
    and quantized fp8e4m3 (plus a /16-scaled copy) for DoubleRow matmuls:
    K=256 contraction in one instruction at 0.5 cycles/row.
  - Weights use a scaled residual split W ~= W8 + fp8(16*(W-W8))/16; the /16
    rides on the second input copy, so two DR matmuls accumulating in PSUM
    recover near-bf16 weight precision at half the PE cost of bf16.
  - RoPE via rotated extra projection rows; combine q' = pm*cos + pr*sin:
    two DVE muls (fused with the PSUM eviction) + one Pool add (SBUF only,
    GPSIMD cannot read PSUM).
  - Attention per (y-window, head pair): sim^T = k^T q fp8-DR, one ACT exp
    per pair at [128,2048] grain -> fp8, AV + all-ones denominator matmuls
    packed [av|av|d|d] so one DVE reciprocal + one DVE mult normalize two
    head-pairs; u stays bf16.
  - Output projection bf16; bias injected by a K=1 f32r matmul that seeds
    PSUM, result DMA'd straight from PSUM to HBM (no eviction op).
  - Stripe pipeline: attention(s) pulls projection(s+1)/output-proj(s-1)
    units as PE filler between window pairs.
"""

import numpy as np

HEADS, WIN, DH, DRR = 8, 16, 64, 32
B, C, H, W = 2, 256, 128, 128
NCORES = 8
SPOS = WIN * W          # 2048 positions per stripe
NT = 4                  # 512-wide position tiles
YW = W // WIN           # 8 y-windows per stripe
SC = 16.0               # residual weight scale

_CACHE = {}


def _build():
    import concourse.bass as bass
    import concourse.mybir as mybir
    import concourse.tile as tile
    from contextlib import ExitStack

    f32 = mybir.dt.float32
    f32r = mybir.dt.float32r
    bf16 = mybir.dt.bfloat16
    fp8 = mybir.dt.float8e4
    AF = mybir.ActivationFunctionType
    MUL = mybir.AluOpType.mult
    ADD = mybir.AluOpType.add
    DR = mybir.MatmulPerfMode.DoubleRow

    nc = bass.Bass("TRN2", target_bir_lowering=False, debug=False,
                   num_devices=NCORES)

    # ---- DRAM parameters (packed/quantized on host)
    xs = nc.declare_dram_parameter("xs", [2, 128, 2, SPOS], fp8, isOutput=False)
    xss = nc.declare_dram_parameter("xss", [2, 128, 2, SPOS], fp8, isOutput=False)
    sks = nc.declare_dram_parameter("sks", [2, 128, 2, SPOS], fp8, isOutput=False)
    skss = nc.declare_dram_parameter("skss", [2, 128, 2, SPOS], fp8, isOutput=False)
    cos4 = nc.declare_dram_parameter("cos4", [2, 128, SPOS], bf16, isOutput=False)
    sin4 = nc.declare_dram_parameter("sin4", [2, 128, SPOS], bf16, isOutput=False)
    wqh = nc.declare_dram_parameter("wqh", [128, 2, 768], fp8, isOutput=False)
    wql = nc.declare_dram_parameter("wql", [128, 2, 768], fp8, isOutput=False)
    wkh = nc.declare_dram_parameter("wkh", [128, 2, 768], fp8, isOutput=False)
    wkl = nc.declare_dram_parameter("wkl", [128, 2, 768], fp8, isOutput=False)
    wvh = nc.declare_dram_parameter("wvh", [128, 2, 512], fp8, isOutput=False)
    wvl = nc.declare_dram_parameter("wvl", [128, 2, 512], fp8, isOutput=False)
    woh = nc.declare_dram_parameter("woh", [128, 4, 256], bf16, isOutput=False)
    bo = nc.declare_dram_parameter("bo", [1, 256], f32r, isOutput=False)
    onesd = nc.declare_dram_parameter("onesd", [1, 512], f32r, isOutput=False)
    out = nc.declare_dram_parameter("out", [2, 2, 128, SPOS], f32, isOutput=True)

    with tile.TileContext(nc) as tc:
        with ExitStack() as es:
            constp = es.enter_context(tc.tile_pool(name="const", bufs=1))
            inp = es.enter_context(tc.tile_pool(name="inp", bufs=1))
            slabp = es.enter_context(tc.tile_pool(name="slab", bufs=1))
            tmpp = es.enter_context(tc.tile_pool(name="tmp", bufs=3))
            rtp = es.enter_context(tc.tile_pool(name="rtp", bufs=2))
            expp = es.enter_context(tc.tile_pool(name="expp", bufs=2))
            outp = es.enter_context(tc.tile_pool(name="outp", bufs=1))
            psim = es.enter_context(tc.tile_pool(name="psim", bufs=1, space="PSUM"))
            pav = es.enter_context(tc.tile_pool(name="pav", bufs=1, space="PSUM"))
            pwork = es.enter_context(tc.tile_pool(name="pwork", bufs=2, space="PSUM"))

            # ---- constants
            def wtile(dram, shape, dt, name):
                t = constp.tile(shape, dt, tag=name, name=name)
                nc.sync.dma_start(out=t[:], in_=dram[:])
                return t

            wq_h = wtile(wqh, [128, 2, 768], fp8, "wqh")
            wq_l = wtile(wql, [128, 2, 768], fp8, "wql")
            wk_h = wtile(wkh, [128, 2, 768], fp8, "wkh")
            wk_l = wtile(wkl, [128, 2, 768], fp8, "wkl")
            wv_h = wtile(wvh, [128, 2, 512], fp8, "wvh")
            wv_l = wtile(wvl, [128, 2, 512], fp8, "wvl")
            wo_t = wtile(woh, [128, 4, 256], bf16, "woh")
            bo_r = wtile(bo, [1, 256], f32r, "bor")
            onerow = wtile(onesd, [1, 512], f32r, "onerow")
            ones_e = constp.tile([128, 2, 128], fp8, tag="onese", name="ones_e")
            nc.gpsimd.memset(ones_e[:], 0.0)
            nc.gpsimd.memset(ones_e[:, :, 0:64], 1.0)
            ones_o = constp.tile([128, 2, 128], fp8, tag="oneso", name="ones_o")
            nc.gpsimd.memset(ones_o[:], 0.0)
            nc.gpsimd.memset(ones_o[:, :, 64:128], 1.0)

            # ---- per-stripe input tiles
            x_t, xl_t, sk_t, skl_t, cos_t, sin_t = {}, {}, {}, {}, {}, {}
            for s in (0, 1):
                x_t[s] = inp.tile([128, 2, SPOS], fp8, tag=f"x{s}", name=f"x{s}")
                xl_t[s] = inp.tile([128, 2, SPOS], fp8, tag=f"xl{s}", name=f"xl{s}")
                sk_t[s] = inp.tile([128, 2, SPOS], fp8, tag=f"sk{s}", name=f"sk{s}")
                skl_t[s] = inp.tile([128, 2, SPOS], fp8, tag=f"skl{s}", name=f"skl{s}")
                cos_t[s] = inp.tile([128, SPOS], bf16, tag=f"cos{s}", name=f"cos{s}")
                sin_t[s] = inp.tile([128, SPOS], bf16, tag=f"sin{s}", name=f"sin{s}")

            def dma_inputs(s):
                nc.sync.dma_start(out=x_t[s][:], in_=xs[s])
                nc.sync.dma_start(out=xl_t[s][:], in_=xss[s])
                nc.sync.dma_start(out=sk_t[s][:], in_=sks[s])
                nc.sync.dma_start(out=skl_t[s][:], in_=skss[s])
                nc.sync.dma_start(out=cos_t[s][:], in_=cos4[s])
                nc.sync.dma_start(out=sin_t[s][:], in_=sin4[s])

            # ---- per-stripe slabs
            q_sl, k_sl, ks_sl, vx_sl, vs_sl, u_sl = {}, {}, {}, {}, {}, {}
            for s in (0, 1):
                for g in (0, 1):
                    q_sl[s, g] = slabp.tile([128, 2, SPOS], fp8, tag=f"q{s}{g}", name=f"q{s}{g}")
                    k_sl[s, g] = slabp.tile([128, 2, SPOS], fp8, tag=f"k{s}{g}", name=f"k{s}{g}")
                    ks_sl[s, g] = slabp.tile([128, 2, SPOS], fp8, tag=f"ks{s}{g}", name=f"ks{s}{g}")
                vx_sl[s] = slabp.tile([128, 16, 768], fp8, tag=f"vx{s}", name=f"vx{s}")
                vs_sl[s] = slabp.tile([128, 16, 768], fp8, tag=f"vs{s}", name=f"vs{s}")
                for vt in (vx_sl[s], vs_sl[s]):
                    zv = vt[:].rearrange("p c (pr t x) -> p (c pr t) x", pr=4, t=3, x=64)
                    nc.gpsimd.memset(zv[:, 1::3, :], 0.0)
                u_sl[s] = slabp.tile([128, 4, SPOS], bf16, tag=f"u{s}", name=f"u{s}")

            # chunk column offsets in the 768-col weight tiles
            PM = {0: 0, 1: 128}
            PASS = {0: 256, 1: 384}
            ROT = {0: 512, 1: 640}

            def proj_units(s):
                """Generator: projections of stripe s in small tile units."""
                for nt in range(NT):
                    ntsl = slice(nt * 512, (nt + 1) * 512)
                    for pi, (wh, wl, rhs_h, rhs_l, dsts) in enumerate((
                        (wq_h, wq_l, x_t[s], xl_t[s], q_sl),
                        (wk_h, wk_l, x_t[s], xl_t[s], k_sl),
                        (wk_h, wk_l, sk_t[s], skl_t[s], ks_sl),
                    )):
                        for g in (0, 1):
                            pm = pwork.tile([128, 512], f32, tag="pw", name="pm")
                            nc.tensor.matmul(pm[:], wh[:, :, PM[g]:PM[g] + 128],
                                             rhs_h[:, :, ntsl], start=True, stop=False,
                                             perf_mode=DR)
                            nc.tensor.matmul(pm[:], wl[:, :, PM[g]:PM[g] + 128],
                                             rhs_l[:, :, ntsl], start=False, stop=True,
                                             perf_mode=DR)
                            pr = pwork.tile([128, 512], f32, tag="pw", name="pr")
                            nc.tensor.matmul(pr[:], wh[:, :, ROT[g]:ROT[g] + 128],
                                             rhs_h[:, :, ntsl], start=True, stop=False,
                                             perf_mode=DR)
                            nc.tensor.matmul(pr[:], wl[:, :, ROT[g]:ROT[g] + 128],
                                             rhs_l[:, :, ntsl], start=False, stop=True,
                                             perf_mode=DR)
                            t1 = tmpp.tile([128, 512], bf16, tag="t1", name="t1")
                            nc.vector.tensor_tensor(out=t1[:], in0=pm[:],
                                                    in1=cos_t[s][:, ntsl], op=MUL)
                            t2 = tmpp.tile([128, 512], bf16, tag="t2", name="t2")
                            nc.vector.tensor_tensor(out=t2[:], in0=pr[:],
                                                    in1=sin_t[s][:, ntsl], op=MUL)
                            nc.gpsimd.tensor_tensor(out=dsts[s, g][:, 0, ntsl],
                                                    in0=t1[:], in1=t2[:], op=ADD)
                            yield
                            pp = pwork.tile([128, 512], f32, tag="pw", name="pp")
                            nc.tensor.matmul(pp[:], wh[:, :, PASS[g]:PASS[g] + 128],
                                             rhs_h[:, :, ntsl], start=True, stop=False,
                                             perf_mode=DR)
                            nc.tensor.matmul(pp[:], wl[:, :, PASS[g]:PASS[g] + 128],
                                             rhs_l[:, :, ntsl], start=False, stop=True,
                                             perf_mode=DR)
                            nc.vector.tensor_copy(dsts[s, g][:, 1, ntsl], pp[:])
                            yield
                    # V projections (transposed orientation), 4 pos-chunks/nt/half
                    for hi, (src_h, src_l, vdst) in enumerate(((x_t[s], xl_t[s], vx_sl[s]),
                                                               (sk_t[s], skl_t[s], vs_sl[s]))):
                        for pc in range(4):
                            cidx = nt * 4 + pc
                            psl = slice(nt * 512 + pc * 128, nt * 512 + pc * 128 + 128)
                            pv = pwork.tile([128, 512], f32, tag="pw", name="pv")
                            nc.tensor.matmul(pv[:], src_h[:, :, psl], wv_h[:],
                                             start=True, stop=False, perf_mode=DR)
                            nc.tensor.matmul(pv[:], src_l[:, :, psl], wv_l[:],
                                             start=False, stop=True, perf_mode=DR)
                            vo = vdst[:, cidx, :].rearrange(
                                "p (pr t x) -> p pr t x", pr=4, t=3, x=64)[:, :, ::2, :]
                            vi = pv[:].rearrange("p (pr t x) -> p pr t x",
                                                 pr=4, t=2, x=64)
                            nc.scalar.copy(vo, vi)
                            yield

            def outproj_units(s):
                """Output projection of stripe s; bias seeded via K=1 matmul,
                result DMA'd straight from PSUM."""
                for nt in range(NT):
                    ntsl = slice(nt * 512, (nt + 1) * 512)
                    for m in (0, 1):
                        fp = pwork.tile([128, 512], f32, tag="pw", name="fp")
                        nc.tensor.matmul(fp[:], bo_r[:, m * 128:(m + 1) * 128],
                                         onerow[:], start=True, stop=False)
                        for kt in range(4):
                            nc.tensor.matmul(fp[:], wo_t[:, kt, m * 128:(m + 1) * 128],
                                             u_sl[s][:, kt, ntsl],
                                             start=False, stop=(kt == 3))
                        osb = outp.tile([128, 512], f32, tag="ot", name="osb")
                        nc.vector.tensor_copy(osb[:], fp[:])
                        nc.sync.dma_start(out=out[s, m, :, ntsl], in_=osb[:])
                        yield

            def emit_sim(s, y, j):
                """sim for window pair (y, 2j), (y, 2j+1); returns exp tile."""
                st = psim.tile([128, 8, 256], f32, tag="sim", name="sim")
                for o, h in ((0, 2 * j), (1, 2 * j + 1)):
                    g, po = h // 4, (h % 4) * 32
                    for mc in range(4):
                        ksl = (k_sl if mc < 2 else ks_sl)[s, g]
                        lhsT = ksl[po:po + 32, :,
                                   y * 256 + (mc % 2) * 128:
                                   y * 256 + (mc % 2) * 128 + 128]
                        rhs = q_sl[s, g][po:po + 32, :, y * 256:y * 256 + 256]
                        nc.tensor.matmul(st[:, 4 * o + mc, :], lhsT, rhs,
                                         start=True, stop=True, perf_mode=DR,
                                         tile_position=(po, 0))
                et = expp.tile([128, 8, 256], fp8, tag="exp", name="exp")
                return st, et

            def emit_exp(st, et):
                nc.scalar.activation(et[:], st[:], AF.Exp, scale=float(DH) ** -0.5)

            def emit_av_batch(s, y, jb, et0, et1):
                """AV + denominator for pairs (y, jb), (y, jb+1); one recip +
                one mult normalize both. pav layout: [av0 av1 d0 d1] x 256."""
                at = pav.tile([128, 1024], f32, tag="av", name="av")
                for bi, (j, et) in enumerate(((jb, et0), (jb + 1, et1))):
                    # windows (y, 2j) -> rows 0:64 and (y, 2j+1) -> rows 64:128
                    # via zero-padded lhsT; 4 matmuls accumulate one region.
                    step = 0
                    for o in (0, 1):
                        pb = j * 192 + o * 64
                        for mc, vsl in ((0, vx_sl[s]), (1, vs_sl[s])):
                            nc.tensor.matmul(at[:, bi * 256:bi * 256 + 256],
                                             vsl[:, 2 * y:2 * y + 2, pb:pb + 128],
                                             et[:, 4 * o + 2 * mc:4 * o + 2 * mc + 2, :],
                                             start=(step == 0), stop=(step == 3),
                                             perf_mode=DR)
                            step += 1
                    step = 0
                    for o, ot in ((0, ones_e), (1, ones_o)):
                        for mc in (0, 1):
                            nc.tensor.matmul(at[:, 512 + bi * 256:512 + bi * 256 + 256],
                                             ot[:],
                                             et[:, 4 * o + 2 * mc:4 * o + 2 * mc + 2, :],
                                             start=(step == 0), stop=(step == 3),
                                             perf_mode=DR)
                            step += 1
                rt = rtp.tile([128, 512], f32, tag="rt", name="rt")
                with nc.allow_low_precision(reason="softmax denominator reciprocal"):
                    nc.vector.reciprocal(rt[:], at[:, 512:1024])
                nc.vector.tensor_tensor(
                    out=u_sl[s][:, jb:jb + 2, y * 256:y * 256 + 256],
                    in0=at[:, 0:512], in1=rt[:], op=MUL)

            def attn_phase(s, filler, pulls, on_y_done=None):
                prev = None
                def flush_prev():
                    nonlocal prev
                    if prev is not None:
                        emit_av_batch(s, *prev)
                        if prev[1] == 2 and on_y_done is not None:
                            on_y_done(prev[0])
                        prev = None
                for y in range(YW):
                    for jb in (0, 2):
                        st0, et0 = emit_sim(s, y, jb)
                        for _ in range(pulls):
                            next(filler, None)
                        emit_exp(st0, et0)
                        st1, et1 = emit_sim(s, y, jb + 1)
                        for _ in range(pulls):
                            next(filler, None)
                        emit_exp(st1, et1)
                        flush_prev()
                        prev = (y, jb, et0, et1)
                flush_prev()
                for _ in range(4):
                    next(filler, None)

            def drain(gen):
                for _ in gen:
                    pass

            # ---- pipeline
            import itertools
            dma_inputs(0)
            f0 = proj_units(0)
            for _ in range(20):           # prologue: stripe-0 nt0
                next(f0)
            dma_inputs(1)
            fill_a = itertools.chain(f0, proj_units(1))
            def trailer(op_gen):
                def y_done(y):
                    if y % 2 == 1:
                        next(op_gen, None)
                        next(op_gen, None)
                return y_done
            op0, op1 = outproj_units(0), outproj_units(1)
            attn_phase(0, fill_a, pulls=4, on_y_done=trailer(op0))
            drain(op0)
            attn_phase(1, fill_a, pulls=3, on_y_done=trailer(op1))
            drain(op1)

    _split_excess_waits(nc)
    return nc


def _split_excess_waits(nc, max_waits=1):
    """walrus accepts one sync-wait command per instruction; hoist excess
    waits onto same-engine NoOps inserted just before."""
    import bass_rust
    import concourse.mybir as mybir
    n_added = 0
    for f in nc.m.functions:
        for bb in f.blocks:
            insts = list(bb.instructions)
            new = []
            dirty = False
            for inst in insts:
                si = inst.sync_info
                if si is not None and len(si.on_wait) > max_waits:
                    waits = list(si.on_wait)
                    for wt in waits[:-max_waits]:
                        nop = mybir.InstNoOp(name=f"{inst.name}-ws{n_added}",
                                             ins=[], outs=[])
                        nop.engine = inst.engine
                        nop.sync_info = bass_rust.SyncInfo(on_wait=[wt], on_update=[])
                        new.append(nop)
                        n_added += 1
                    inst.sync_info = bass_rust.SyncInfo(
                        on_wait=waits[-max_waits:], on_update=list(si.on_update))
                    dirty = True
                new.append(inst)
            if dirty:
                bb.instructions = new
    return n_added


def _window_major(a):
    """[..., 16, 128] spatial block -> [..., 2048] window-major positions."""
    lead = a.shape[:-2]
    return (a.reshape(*lead, WIN, YW, WIN)
             .swapaxes(-3, -2)
             .reshape(*lead, SPOS))


def _pack_kt(a):
    """[256, N] -> [128, 2, N] (channel c = kt*128 + p)."""
    n = a.shape[-1]
    return np.ascontiguousarray(a.reshape(2, 128, n).transpose(1, 0, 2))


def _rot_weights(Wm):
    R = np.zeros_like(Wm)
    for h in range(HEADS):
        b0 = h * DH
        for i in range(DRR // 2):
            R[b0 + 2 * i] = -Wm[b0 + 2 * i + 1]
            R[b0 + 2 * i + 1] = Wm[b0 + 2 * i]
    return R


def _qk_weight(Wm):
    """[512, 256] -> [768, 256] rows ordered (pm-g0, pm-g1, pass-g0, pass-g1,
    rot-g0, rot-g1), each chunk of 128 = 4 heads x 32 channels."""
    R = _rot_weights(Wm)
    rows = np.empty((768, C), np.float32)
    for g in (0, 1):
        for p in range(128):
            h, d = 4 * g + p // 32, p % 32
            rows[g * 128 + p] = Wm[h * DH + d]              # pm (rope dims)
            rows[256 + g * 128 + p] = Wm[h * DH + 32 + d]   # pass
            rows[512 + g * 128 + p] = R[h * DH + d]         # rot
    return rows


def _make_core_inputs(x, skip, time_emb, sin, cos, Wq, Wkv, Wout, bout):
    import ml_dtypes
    f8 = ml_dtypes.float8_e4m3
    bfd = ml_dtypes.bfloat16

    def split_resid(rows):  # [M, 256] f32 -> hi/lo [128, 2, M] fp8
        hi = np.asarray(rows, f8).astype(np.float32)
        lo = SC * (np.asarray(rows, np.float32) - hi)
        return (np.asarray(_pack_kt(hi.T), f8),
                np.asarray(_pack_kt(np.asarray(lo, f8).astype(np.float32).T), f8))

    Wk_, Wv_ = Wkv[:512], Wkv[512:]
    wqh_a, wql_a = split_resid(_qk_weight(Wq))
    wkh_a, wkl_a = split_resid(_qk_weight(Wk_))
    wvh_a, wvl_a = split_resid(Wv_)      # natural col order h*64+d

    # wo rows: u-channel (p, kt): h = 2*kt + p//64, d = p%64
    wo_rows = np.empty((128, 4, C), np.float32)
    for p in range(128):
        for kt in range(4):
            h, d = 2 * kt + p // 64, p % 64
            wo_rows[p, kt] = Wout[:, h * DH + d]
    woh_a = np.asarray(wo_rows, bfd)
    bo_a = bout.reshape(1, C).astype(np.float32).copy()

    in_maps = []
    for core in range(NCORES):
        b = core // 4
        xbs = [2 * (core % 4), 2 * (core % 4) + 1]
        xs_c = np.empty((2, 128, 2, SPOS), f8)
        xss_c = np.empty((2, 128, 2, SPOS), f8)
        sk_c = np.empty((2, 128, 2, SPOS), f8)
        skss_c = np.empty((2, 128, 2, SPOS), f8)
        cos_c = np.empty((2, 128, SPOS), bfd)
        sin_c = np.empty((2, 128, SPOS), bfd)
        for si, xb in enumerate(xbs):
            rs = slice(xb * WIN, (xb + 1) * WIN)
            xf = _window_major((x[b, :, rs, :] + time_emb[b][:, None, None])
                               .astype(np.float32))
            skf = _window_major(skip[b, :, rs, :].astype(np.float32))
            xs_c[si] = np.asarray(_pack_kt(xf), f8)
            xss_c[si] = np.asarray(_pack_kt(xf / SC), f8)
            sk_c[si] = np.asarray(_pack_kt(skf), f8)
            skss_c[si] = np.asarray(_pack_kt(skf / SC), f8)
            cw = _window_major(cos[rs].transpose(2, 0, 1))   # [32, 2048]
            sw = _window_major(sin[rs].transpose(2, 0, 1))
            cos_c[si] = np.asarray(np.tile(cw, (4, 1)), bfd)
            sin_c[si] = np.asarray(np.tile(sw, (4, 1)), bfd)
        in_maps.append({
            "xs": xs_c, "xss": xss_c, "sks": sk_c, "skss": skss_c,
            "cos4": cos_c, "sin4": sin_c,
            "wqh": wqh_a, "wql": wql_a, "wkh": wkh_a, "wkl": wkl_a,
            "wvh": wvh_a, "wvl": wvl_a, "woh": woh_a, "bo": bo_a,
            "onesd": np.ones((1, 512), np.float32),
        })
    return in_maps


def _assemble(results):
    out_full = np.empty((B, C, H, W), np.float32)
    for core in range(NCORES):
        b = core // 4
        xbs = [2 * (core % 4), 2 * (core % 4) + 1]
        o = results[core]["out"]          # [2, 2, 128, 2048] window-major
        for si, xb in enumerate(xbs):
            ch = o[si].reshape(C, YW, WIN, WIN)
            blk = ch.swapaxes(1, 2).reshape(C, WIN, W)
            out_full[b, :, xb * WIN:(xb + 1) * WIN, :] = blk
    return out_full


def get_nc():
    if "nc" not in _CACHE:
        _CACHE["nc"] = _build()
    return _CACHE["nc"]


def kernel(x, skip, time_emb, sin, cos, Wq, Wkv, Wout, bout):
    from concourse.bass_utils import run_bass_kernel_spmd
    args = [np.asarray(a, dtype=np.float32) for a in
            (x, skip, time_emb, sin, cos, Wq, Wkv, Wout, bout)]
    nc = get_nc()
    in_maps = _make_core_inputs(*args)
    res = run_bass_kernel_spmd(nc, in_maps, list(range(NCORES)), trace=False)
    return _assemble(res.results)


# revision 16
# speedup vs baseline: 1.0348x; 1.0348x over previous
"""Trainium2 Bass kernel for nn_Attention_6201932775733 (sparse window attention).

Strategy (8 NeuronCores, SPMD, data-parallel over (batch, row-stripe)):
  - Core i handles batch i//4, two 16-row stripes {2*(i%4), 2*(i%4)+1}.
  - Inputs host-permuted to window-major position order, packed [128, kt, pos]
    and quantized fp8e4m3 (plus a /16-scaled copy) for DoubleRow matmuls:
    K=256 contraction in one instruction at 0.5 cycles/row.
  - Weights use a scaled residual split W ~= W8 + fp8(16*(W-W8))/16; the /16
    rides on the second input copy, so two DR matmuls accumulating in PSUM
    recover near-bf16 weight precision at half the PE cost of bf16.
  - RoPE via rotated extra projection rows; combine q' = pm*cos + pr*sin:
    two DVE muls (fused with the PSUM eviction) + one Pool add (SBUF only,
    GPSIMD cannot read PSUM).
  - Attention per (y-window, head pair): sim^T = k^T q fp8-DR, one ACT exp
    per pair at [128,2048] grain -> fp8, AV + all-ones denominator matmuls
    packed [av|av|d|d] so one DVE reciprocal + one DVE mult normalize two
    head-pairs; u stays bf16.
  - Output projection bf16; bias injected by a K=1 f32r matmul that seeds
    PSUM, result DMA'd straight from PSUM to HBM (no eviction op).
  - Stripe pipeline: attention(s) pulls projection(s+1)/output-proj(s-1)
    units as PE filler between window pairs.
"""

import numpy as np

HEADS, WIN, DH, DRR = 8, 16, 64, 32
B, C, H, W = 2, 256, 128, 128
NCORES = 8
SPOS = WIN * W          # 2048 positions per stripe
NT = 4                  # 512-wide position tiles
YW = W // WIN           # 8 y-windows per stripe
SC = 16.0               # residual weight scale

_CACHE = {}


def _build():
    import concourse.bass as bass
    import concourse.mybir as mybir
    import concourse.tile as tile
    from contextlib import ExitStack

    f32 = mybir.dt.float32
    f32r = mybir.dt.float32r
    bf16 = mybir.dt.bfloat16
    fp8 = mybir.dt.float8e4
    AF = mybir.ActivationFunctionType
    MUL = mybir.AluOpType.mult
    ADD = mybir.AluOpType.add
    DR = mybir.MatmulPerfMode.DoubleRow

    nc = bass.Bass("TRN2", target_bir_lowering=False, debug=False,
                   num_devices=NCORES)

    # ---- DRAM parameters (packed/quantized on host)
    xs = nc.declare_dram_parameter("xs", [2, 128, 2, SPOS], fp8, isOutput=False)
    xss = nc.declare_dram_parameter("xss", [2, 128, 2, SPOS], fp8, isOutput=False)
    sks = nc.declare_dram_parameter("sks", [2, 128, 2, SPOS], fp8, isOutput=False)
    skss = nc.declare_dram_parameter("skss", [2, 128, 2, SPOS], fp8, isOutput=False)
    cos4 = nc.declare_dram_parameter("cos4", [2, 128, SPOS], bf16, isOutput=False)
    sin4 = nc.declare_dram_parameter("sin4", [2, 128, SPOS], bf16, isOutput=False)
    wqh = nc.declare_dram_parameter("wqh", [128, 2, 768], fp8, isOutput=False)
    wql = nc.declare_dram_parameter("wql", [128, 2, 768], fp8, isOutput=False)
    wkh = nc.declare_dram_parameter("wkh", [128, 2, 768], fp8, isOutput=False)
    wkl = nc.declare_dram_parameter("wkl", [128, 2, 768], fp8, isOutput=False)
    wvh = nc.declare_dram_parameter("wvh", [128, 2, 512], fp8, isOutput=False)
    wvl = nc.declare_dram_parameter("wvl", [128, 2, 512], fp8, isOutput=False)
    woh = nc.declare_dram_parameter("woh", [128, 4, 256], bf16, isOutput=False)
    bo = nc.declare_dram_parameter("bo", [1, 256], f32r, isOutput=False)
    onesd = nc.declare_dram_parameter("onesd", [1, 512], f32r, isOutput=False)
    out = nc.declare_dram_parameter("out", [2, 2, 128, SPOS], f32, isOutput=True)

    with tile.TileContext(nc) as tc:
        with ExitStack() as es:
            constp = es.enter_context(tc.tile_pool(name="const", bufs=1))
            inp = es.enter_context(tc.tile_pool(name="inp", bufs=1))
            slabp = es.enter_context(tc.tile_pool(name="slab", bufs=1))
            tmpp = es.enter_context(tc.tile_pool(name="tmp", bufs=3))
            rtp = es.enter_context(tc.tile_pool(name="rtp", bufs=2))
            expp = es.enter_context(tc.tile_pool(name="expp", bufs=2))
            outp = es.enter_context(tc.tile_pool(name="outp", bufs=1))
            psim = es.enter_context(tc.tile_pool(name="psim", bufs=1, space="PSUM"))
            pav = es.enter_context(tc.tile_pool(name="pav", bufs=1, space="PSUM"))
            pwork = es.enter_context(tc.tile_pool(name="pwork", bufs=2, space="PSUM"))

            # ---- constants
            def wtile(dram, shape, dt, name):
                t = constp.tile(shape, dt, tag=name, name=name)
                nc.sync.dma_start(out=t[:], in_=dram[:])
                return t

            wq_h = wtile(wqh, [128, 2, 768], fp8, "wqh")
            wq_l = wtile(wql, [128, 2, 768], fp8, "wql")
            wk_h = wtile(wkh, [128, 2, 768], fp8, "wkh")
            wk_l = wtile(wkl, [128, 2, 768], fp8, "wkl")
            wv_h = wtile(wvh, [128, 2, 512], fp8, "wvh")
            wv_l = wtile(wvl, [128, 2, 512], fp8, "wvl")
            wo_t = wtile(woh, [128, 4, 256], bf16, "woh")
            bo_r = wtile(bo, [1, 256], f32r, "bor")
            onerow = wtile(onesd, [1, 512], f32r, "onerow")
            ones_e = constp.tile([128, 2, 128], fp8, tag="onese", name="ones_e")
            nc.gpsimd.memset(ones_e[:], 0.0)
            nc.gpsimd.memset(ones_e[:, :, 0:64], 1.0)
            ones_o = constp.tile([128, 2, 128], fp8, tag="oneso", name="ones_o")
            nc.gpsimd.memset(ones_o[:], 0.0)
            nc.gpsimd.memset(ones_o[:, :, 64:128], 1.0)

            # ---- per-stripe input tiles
            x_t, xl_t, sk_t, skl_t, cos_t, sin_t = {}, {}, {}, {}, {}, {}
            for s in (0, 1):
                x_t[s] = inp.tile([128, 2, SPOS], fp8, tag=f"x{s}", name=f"x{s}")
                xl_t[s] = inp.tile([128, 2, SPOS], fp8, tag=f"xl{s}", name=f"xl{s}")
                sk_t[s] = inp.tile([128, 2, SPOS], fp8, tag=f"sk{s}", name=f"sk{s}")
                skl_t[s] = inp.tile([128, 2, SPOS], fp8, tag=f"skl{s}", name=f"skl{s}")
                cos_t[s] = inp.tile([128, SPOS], bf16, tag=f"cos{s}", name=f"cos{s}")
                sin_t[s] = inp.tile([128, SPOS], bf16, tag=f"sin{s}", name=f"sin{s}")

            def dma_inputs(s):
                nc.sync.dma_start(out=x_t[s][:], in_=xs[s])
                nc.sync.dma_start(out=xl_t[s][:], in_=xss[s])
                nc.sync.dma_start(out=sk_t[s][:], in_=sks[s])
                nc.sync.dma_start(out=skl_t[s][:], in_=skss[s])
                nc.sync.dma_start(out=cos_t[s][:], in_=cos4[s])
                nc.sync.dma_start(out=sin_t[s][:], in_=sin4[s])

            # ---- per-stripe slabs
            q_sl, k_sl, ks_sl, vx_sl, vs_sl, u_sl = {}, {}, {}, {}, {}, {}
            for s in (0, 1):
                for g in (0, 1):
                    q_sl[s, g] = slabp.tile([128, 2, SPOS], fp8, tag=f"q{s}{g}", name=f"q{s}{g}")
                    k_sl[s, g] = slabp.tile([128, 2, SPOS], fp8, tag=f"k{s}{g}", name=f"k{s}{g}")
                    ks_sl[s, g] = slabp.tile([128, 2, SPOS], fp8, tag=f"ks{s}{g}", name=f"ks{s}{g}")
                vx_sl[s] = slabp.tile([128, 16, 768], fp8, tag=f"vx{s}", name=f"vx{s}")
                vs_sl[s] = slabp.tile([128, 16, 768], fp8, tag=f"vs{s}", name=f"vs{s}")
                for vt in (vx_sl[s], vs_sl[s]):
                    zv = vt[:].rearrange("p c (pr t x) -> p (c pr t) x", pr=4, t=3, x=64)
                    nc.gpsimd.memset(zv[:, 1::3, :], 0.0)
                u_sl[s] = slabp.tile([128, 4, SPOS], bf16, tag=f"u{s}", name=f"u{s}")

            # chunk column offsets in the 768-col weight tiles
            PM = {0: 0, 1: 128}
            PASS = {0: 256, 1: 384}
            ROT = {0: 512, 1: 640}

            def proj_units(s):
                """Generator: projections of stripe s in small tile units."""
                for nt in range(NT):
                    ntsl = slice(nt * 512, (nt + 1) * 512)
                    for pi, (wh, wl, rhs_h, rhs_l, dsts) in enumerate((
                        (wq_h, wq_l, x_t[s], xl_t[s], q_sl),
                        (wk_h, wk_l, x_t[s], xl_t[s], k_sl),
                        (wk_h, wk_l, sk_t[s], skl_t[s], ks_sl),
                    )):
                        for g in (0, 1):
                            pm = pwork.tile([128, 512], f32, tag="pw", name="pm")
                            nc.tensor.matmul(pm[:], wh[:, :, PM[g]:PM[g] + 128],
                                             rhs_h[:, :, ntsl], start=True, stop=False,
                                             perf_mode=DR)
                            nc.tensor.matmul(pm[:], wl[:, :, PM[g]:PM[g] + 128],
                                             rhs_l[:, :, ntsl], start=False, stop=True,
                                             perf_mode=DR)
                            pr = pwork.tile([128, 512], f32, tag="pw", name="pr")
                            nc.tensor.matmul(pr[:], wh[:, :, ROT[g]:ROT[g] + 128],
                                             rhs_h[:, :, ntsl], start=True, stop=False,
                                             perf_mode=DR)
                            nc.tensor.matmul(pr[:], wl[:, :, ROT[g]:ROT[g] + 128],
                                             rhs_l[:, :, ntsl], start=False, stop=True,
                                             perf_mode=DR)
                            t1 = tmpp.tile([128, 512], bf16, tag="t1", name="t1")
                            nc.vector.tensor_tensor(out=t1[:], in0=pm[:],
                                                    in1=cos_t[s][:, ntsl], op=MUL)
                            t2 = tmpp.tile([128, 512], bf16, tag="t2", name="t2")
                            nc.vector.tensor_tensor(out=t2[:], in0=pr[:],
                                                    in1=sin_t[s][:, ntsl], op=MUL)
                            nc.gpsimd.tensor_tensor(out=dsts[s, g][:, 0, ntsl],
                                                    in0=t1[:], in1=t2[:], op=ADD)
                            yield
                            pp = pwork.tile([128, 512], f32, tag="pw", name="pp")
                            nc.tensor.matmul(pp[:], wh[:, :, PASS[g]:PASS[g] + 128],
                                             rhs_h[:, :, ntsl], start=True, stop=False,
                                             perf_mode=DR)
                            nc.tensor.matmul(pp[:], wl[:, :, PASS[g]:PASS[g] + 128],
                                             rhs_l[:, :, ntsl], start=False, stop=True,
                                             perf_mode=DR)
                            nc.vector.tensor_copy(dsts[s, g][:, 1, ntsl], pp[:])
                            yield
                    # V projections (transposed orientation), 4 pos-chunks/nt/half
                    for hi, (src_h, src_l, vdst) in enumerate(((x_t[s], xl_t[s], vx_sl[s]),
                                                               (sk_t[s], skl_t[s], vs_sl[s]))):
                        for pc in range(4):
                            cidx = nt * 4 + pc
                            psl = slice(nt * 512 + pc * 128, nt * 512 + pc * 128 + 128)
                            pv = pwork.tile([128, 512], f32, tag="pw", name="pv")
                            nc.tensor.matmul(pv[:], src_h[:, :, psl], wv_h[:],
                                             start=True, stop=False, perf_mode=DR)
                            nc.tensor.matmul(pv[:], src_l[:, :, psl], wv_l[:],
                                             start=False, stop=True, perf_mode=DR)
                            vo = vdst[:, cidx, :].rearrange(
                                "p (pr t x) -> p pr t x", pr=4, t=3, x=64)[:, :, ::2, :]
                            vi = pv[:].rearrange("p (pr t x) -> p pr t x",
                                                 pr=4, t=2, x=64)
                            nc.scalar.copy(vo, vi)
                            yield

            def outproj_units(s):
                """Output projection of stripe s; bias seeded via K=1 matmul,
                result DMA'd straight from PSUM."""
                for nt in range(NT):
                    ntsl = slice(nt * 512, (nt + 1) * 512)
                    for m in (0, 1):
                        fp = pwork.tile([128, 512], f32, tag="pw", name="fp")
                        nc.tensor.matmul(fp[:], bo_r[:, m * 128:(m + 1) * 128],
                                         onerow[:], start=True, stop=False)
                        for kt in range(4):
                            nc.tensor.matmul(fp[:], wo_t[:, kt, m * 128:(m + 1) * 128],
                                             u_sl[s][:, kt, ntsl],
                                             start=False, stop=(kt == 3))
                        osb = outp.tile([128, 512], f32, tag="ot", name="osb")
                        nc.vector.tensor_copy(osb[:], fp[:])
                        nc.sync.dma_start(out=out[s, m, :, ntsl], in_=osb[:])
                        yield

            def emit_sim(s, y, j):
                """sim for window pair (y, 2j), (y, 2j+1); returns exp tile."""
                st = psim.tile([128, 8, 256], f32, tag="sim", name="sim")
                for o, h in ((0, 2 * j), (1, 2 * j + 1)):
                    g, po = h // 4, (h % 4) * 32
                    for mc in range(4):
                        ksl = (k_sl if mc < 2 else ks_sl)[s, g]
                        lhsT = ksl[po:po + 32, :,
                                   y * 256 + (mc % 2) * 128:
                                   y * 256 + (mc % 2) * 128 + 128]
                        rhs = q_sl[s, g][po:po + 32, :, y * 256:y * 256 + 256]
                        nc.tensor.matmul(st[:, 4 * o + mc, :], lhsT, rhs,
                                         start=True, stop=True, perf_mode=DR,
                                         tile_position=(po, 0))
                et = expp.tile([128, 8, 256], fp8, tag="exp", name="exp")
                return st, et

            def emit_exp(st, et):
                nc.scalar.activation(et[:], st[:], AF.Exp, scale=float(DH) ** -0.5)

            def emit_av_batch(s, y, jb, et0, et1):
                """AV + denominator for pairs (y, jb), (y, jb+1); one recip +
                one mult normalize both. pav layout: [av0 av1 d0 d1] x 256."""
                at = pav.tile([128, 1024], f32, tag="av", name="av")
                for bi, (j, et) in enumerate(((jb, et0), (jb + 1, et1))):
                    # windows (y, 2j) -> rows 0:64 and (y, 2j+1) -> rows 64:128
                    # via zero-padded lhsT; 4 matmuls accumulate one region.
                    step = 0
                    for o in (0, 1):
                        pb = j * 192 + o * 64
                        for mc, vsl in ((0, vx_sl[s]), (1, vs_sl[s])):
                            nc.tensor.matmul(at[:, bi * 256:bi * 256 + 256],
                                             vsl[:, 2 * y:2 * y + 2, pb:pb + 128],
                                             et[:, 4 * o + 2 * mc:4 * o + 2 * mc + 2, :],
                                             start=(step == 0), stop=(step == 3),
                                             perf_mode=DR)
                            step += 1
                    step = 0
                    for o, ot in ((0, ones_e), (1, ones_o)):
                        for mc in (0, 1):
                            nc.tensor.matmul(at[:, 512 + bi * 256:512 + bi * 256 + 256],
                                             ot[:],
                                             et[:, 4 * o + 2 * mc:4 * o + 2 * mc + 2, :],
                                             start=(step == 0), stop=(step == 3),
                                             perf_mode=DR)
                            step += 1
                rt = rtp.tile([128, 512], f32, tag="rt", name="rt")
                with nc.allow_low_precision(reason="softmax denominator reciprocal"):
                    nc.vector.reciprocal(rt[:], at[:, 512:1024])
                nc.vector.tensor_tensor(
                    out=u_sl[s][:, jb:jb + 2, y * 256:y * 256 + 256],
                    in0=at[:, 0:512], in1=rt[:], op=MUL)

            def attn_phase(s, filler, pulls, on_y_done=None):
                prev = None
                def flush_prev():
                    nonlocal prev
                    if prev is not None:
                        emit_av_batch(s, *prev)
                        if prev[1] == 2 and on_y_done is not None:
                            on_y_done(prev[0])
                        prev = None
                for y in range(YW):
                    for jb in (0, 2):
                        st0, et0 = emit_sim(s, y, jb)
                        for _ in range(pulls):
                            next(filler, None)
                        emit_exp(st0, et0)
                        st1, et1 = emit_sim(s, y, jb + 1)
                        for _ in range(pulls):
                            next(filler, None)
                        emit_exp(st1, et1)
                        flush_prev()
                        prev = (y, jb, et0, et1)
                flush_prev()
                for _ in range(4):
                    next(filler, None)

            def drain(gen):
                for _ in gen:
                    pass

            # ---- pipeline
            dma_inputs(0)
            dma_inputs(1)
            drain(proj_units(0))
            attn_phase(0, proj_units(1), pulls=3)
            attn_phase(1, outproj_units(0), pulls=1)
            drain(outproj_units(1))

    _split_excess_waits(nc)
    return nc


def _split_excess_waits(nc, max_waits=1):
    """walrus accepts one sync-wait command per instruction; hoist excess
    waits onto same-engine NoOps inserted just before."""
    import bass_rust
    import concourse.mybir as mybir
    n_added = 0
    for f in nc.m.functions:
        for bb in f.blocks:
            insts = list(bb.instructions)
            new = []
            dirty = False
            for inst in insts:
                si = inst.sync_info
                if si is not None and len(si.on_wait) > max_waits:
                    waits = list(si.on_wait)
                    for wt in waits[:-max_waits]:
                        nop = mybir.InstNoOp(name=f"{inst.name}-ws{n_added}",
                                             ins=[], outs=[])
                        nop.engine = inst.engine
                        nop.sync_info = bass_rust.SyncInfo(on_wait=[wt], on_update=[])
                        new.append(nop)
                        n_added += 1
                    inst.sync_info = bass_rust.SyncInfo(
                        on_wait=waits[-max_waits:], on_update=list(si.on_update))
                    dirty = True
                new.append(inst)
            if dirty:
                bb.instructions = new
    return n_added


def _window_major(a):
    """[..., 16, 128] spatial block -> [..., 2048] window-major positions."""
    lead = a.shape[:-2]
    return (a.reshape(*lead, WIN, YW, WIN)
             .swapaxes(-3, -2)
             .reshape(*lead, SPOS))


def _pack_kt(a):
    """[256, N] -> [128, 2, N] (channel c = kt*128 + p)."""
    n = a.shape[-1]
    return np.ascontiguousarray(a.reshape(2, 128, n).transpose(1, 0, 2))


def _rot_weights(Wm):
    R = np.zeros_like(Wm)
    for h in range(HEADS):
        b0 = h * DH
        for i in range(DRR // 2):
            R[b0 + 2 * i] = -Wm[b0 + 2 * i + 1]
            R[b0 + 2 * i + 1] = Wm[b0 + 2 * i]
    return R


def _qk_weight(Wm):
    """[512, 256] -> [768, 256] rows ordered (pm-g0, pm-g1, pass-g0, pass-g1,
    rot-g0, rot-g1), each chunk of 128 = 4 heads x 32 channels."""
    R = _rot_weights(Wm)
    rows = np.empty((768, C), np.float32)
    for g in (0, 1):
        for p in range(128):
            h, d = 4 * g + p // 32, p % 32
            rows[g * 128 + p] = Wm[h * DH + d]              # pm (rope dims)
            rows[256 + g * 128 + p] = Wm[h * DH + 32 + d]   # pass
            rows[512 + g * 128 + p] = R[h * DH + d]         # rot
    return rows


def _make_core_inputs(x, skip, time_emb, sin, cos, Wq, Wkv, Wout, bout):
    import ml_dtypes
    f8 = ml_dtypes.float8_e4m3
    bfd = ml_dtypes.bfloat16

    def split_resid(rows):  # [M, 256] f32 -> hi/lo [128, 2, M] fp8
        hi = np.asarray(rows, f8).astype(np.float32)
        lo = SC * (np.asarray(rows, np.float32) - hi)
        return (np.asarray(_pack_kt(hi.T), f8),
                np.asarray(_pack_kt(np.asarray(lo, f8).astype(np.float32).T), f8))

    Wk_, Wv_ = Wkv[:512], Wkv[512:]
    wqh_a, wql_a = split_resid(_qk_weight(Wq))
    wkh_a, wkl_a = split_resid(_qk_weight(Wk_))
    wvh_a, wvl_a = split_resid(Wv_)      # natural col order h*64+d

    # wo rows: u-channel (p, kt): h = 2*kt + p//64, d = p%64
    wo_rows = np.empty((128, 4, C), np.float32)
    for p in range(128):
        for kt in range(4):
            h, d = 2 * kt + p // 64, p % 64
            wo_rows[p, kt] = Wout[:, h * DH + d]
    woh_a = np.asarray(wo_rows, bfd)
    bo_a = bout.reshape(1, C).astype(np.float32).copy()

    in_maps = []
    for core in range(NCORES):
        b = core // 4
        xbs = [2 * (core % 4), 2 * (core % 4) + 1]
        xs_c = np.empty((2, 128, 2, SPOS), f8)
        xss_c = np.empty((2, 128, 2, SPOS), f8)
        sk_c = np.empty((2, 128, 2, SPOS), f8)
        skss_c = np.empty((2, 128, 2, SPOS), f8)
        cos_c = np.empty((2, 128, SPOS), bfd)
        sin_c = np.empty((2, 128, SPOS), bfd)
        for si, xb in enumerate(xbs):
            rs = slice(xb * WIN, (xb + 1) * WIN)
            xf = _window_major((x[b, :, rs, :] + time_emb[b][:, None, None])
                               .astype(np.float32))
            skf = _window_major(skip[b, :, rs, :].astype(np.float32))
            xs_c[si] = np.asarray(_pack_kt(xf), f8)
            xss_c[si] = np.asarray(_pack_kt(xf / SC), f8)
            sk_c[si] = np.asarray(_pack_kt(skf), f8)
            skss_c[si] = np.asarray(_pack_kt(skf / SC), f8)
            cw = _window_major(cos[rs].transpose(2, 0, 1))   # [32, 2048]
            sw = _window_major(sin[rs].transpose(2, 0, 1))
            cos_c[si] = np.asarray(np.tile(cw, (4, 1)), bfd)
            sin_c[si] = np.asarray(np.tile(sw, (4, 1)), bfd)
        in_maps.append({
            "xs": xs_c, "xss": xss_c, "sks": sk_c, "skss": skss_c,
            "cos4": cos_c, "sin4": sin_c,
            "wqh": wqh_a, "wql": wql_a, "wkh": wkh_a, "wkl": wkl_a,
            "wvh": wvh_a, "wvl": wvl_a, "woh": woh_a, "bo": bo_a,
            "onesd": np.ones((1, 512), np.float32),
        })
    return in_maps


def _assemble(results):
    out_full = np.empty((B, C, H, W), np.float32)
    for core in range(NCORES):
        b = core // 4
        xbs = [2 * (core % 4), 2 * (core % 4) + 1]
        o = results[core]["out"]          # [2, 2, 128, 2048] window-major
        for si, xb in enumerate(xbs):
            ch = o[si].reshape(C, YW, WIN, WIN)
            blk = ch.swapaxes(1, 2).reshape(C, WIN, W)
            out_full[b, :, xb * WIN:(xb + 1) * WIN, :] = blk
    return out_full


def get_nc():
    if "nc" not in _CACHE:
        _CACHE["nc"] = _build()
    return _CACHE["nc"]


def kernel(x, skip, time_emb, sin, cos, Wq, Wkv, Wout, bout):
    from concourse.bass_utils import run_bass_kernel_spmd
    args = [np.asarray(a, dtype=np.float32) for a in
            (x, skip, time_emb, sin, cos, Wq, Wkv, Wout, bout)]
    nc = get_nc()
    in_maps = _make_core_inputs(*args)
    res = run_bass_kernel_spmd(nc, in_maps, list(range(NCORES)), trace=False)
    return _assemble(res.results)


# revision 18
# speedup vs baseline: 1.0709x; 1.0349x over previous
"""Trainium2 Bass kernel for nn_Attention_6201932775733 (sparse window attention).

Strategy (8 NeuronCores, SPMD, data-parallel over (batch, row-stripe)):
  - Core i handles batch i//4, two 16-row stripes {2*(i%4), 2*(i%4)+1}.
  - Inputs host-permuted to window-major position order, packed [128, kt, pos]
    and quantized fp8e4m3 (plus a /16-scaled copy) for DoubleRow matmuls:
    K=256 contraction in one instruction at 0.5 cycles/row.
  - Weights use a scaled residual split W ~= W8 + fp8(16*(W-W8))/16; the /16
    rides on the second input copy, so two DR matmuls accumulating in PSUM
    recover near-bf16 weight precision at half the PE cost of bf16.
  - RoPE via rotated extra projection rows; combine q' = pm*cos + pr*sin:
    two DVE muls (fused with the PSUM eviction) + one Pool add (SBUF only,
    GPSIMD cannot read PSUM).
  - Attention per (y-window, head pair): sim^T = k^T q fp8-DR, one ACT exp
    per pair at [128,2048] grain -> fp8, AV + all-ones denominator matmuls
    packed [av|av|d|d] so one DVE reciprocal + one DVE mult normalize two
    head-pairs; u stays bf16.
  - Output projection bf16; bias injected by a K=1 f32r matmul that seeds
    PSUM, result DMA'd straight from PSUM to HBM (no eviction op).
  - Stripe pipeline: attention(s) pulls projection(s+1)/output-proj(s-1)
    units as PE filler between window pairs.
"""

import numpy as np

HEADS, WIN, DH, DRR = 8, 16, 64, 32
B, C, H, W = 2, 256, 128, 128
NCORES = 8
SPOS = WIN * W          # 2048 positions per stripe
NT = 4                  # 512-wide position tiles
YW = W // WIN           # 8 y-windows per stripe
SC = 16.0               # residual weight scale

_CACHE = {}


def _build():
    import concourse.bass as bass
    import concourse.mybir as mybir
    import concourse.tile as tile
    from contextlib import ExitStack

    f32 = mybir.dt.float32
    f32r = mybir.dt.float32r
    bf16 = mybir.dt.bfloat16
    fp8 = mybir.dt.float8e4
    AF = mybir.ActivationFunctionType
    MUL = mybir.AluOpType.mult
    ADD = mybir.AluOpType.add
    DR = mybir.MatmulPerfMode.DoubleRow

    nc = bass.Bass("TRN2", target_bir_lowering=False, debug=False,
                   num_devices=NCORES)

    # ---- DRAM parameters (packed/quantized on host)
    xs = nc.declare_dram_parameter("xs", [2, 128, 2, SPOS], fp8, isOutput=False)
    xss = nc.declare_dram_parameter("xss", [2, 128, 2, SPOS], fp8, isOutput=False)
    sks = nc.declare_dram_parameter("sks", [2, 128, 2, SPOS], fp8, isOutput=False)
    skss = nc.declare_dram_parameter("skss", [2, 128, 2, SPOS], fp8, isOutput=False)
    cos4 = nc.declare_dram_parameter("cos4", [2, 128, SPOS], bf16, isOutput=False)
    sin4 = nc.declare_dram_parameter("sin4", [2, 128, SPOS], bf16, isOutput=False)
    wqh = nc.declare_dram_parameter("wqh", [128, 2, 768], fp8, isOutput=False)
    wql = nc.declare_dram_parameter("wql", [128, 2, 768], fp8, isOutput=False)
    wkh = nc.declare_dram_parameter("wkh", [128, 2, 768], fp8, isOutput=False)
    wkl = nc.declare_dram_parameter("wkl", [128, 2, 768], fp8, isOutput=False)
    wvh = nc.declare_dram_parameter("wvh", [128, 2, 512], fp8, isOutput=False)
    wvl = nc.declare_dram_parameter("wvl", [128, 2, 512], fp8, isOutput=False)
    woh = nc.declare_dram_parameter("woh", [128, 4, 256], bf16, isOutput=False)
    bo = nc.declare_dram_parameter("bo", [1, 256], f32r, isOutput=False)
    onesd = nc.declare_dram_parameter("onesd", [1, 512], f32r, isOutput=False)
    out = nc.declare_dram_parameter("out", [2, 2, 128, SPOS], f32, isOutput=True)

    with tile.TileContext(nc) as tc:
        with ExitStack() as es:
            constp = es.enter_context(tc.tile_pool(name="const", bufs=1))
            inp = es.enter_context(tc.tile_pool(name="inp", bufs=1))
            slabp = es.enter_context(tc.tile_pool(name="slab", bufs=1))
            tmpp = es.enter_context(tc.tile_pool(name="tmp", bufs=3))
            rtp = es.enter_context(tc.tile_pool(name="rtp", bufs=2))
            expp = es.enter_context(tc.tile_pool(name="expp", bufs=2))
            outp = es.enter_context(tc.tile_pool(name="outp", bufs=1))
            psim = es.enter_context(tc.tile_pool(name="psim", bufs=1, space="PSUM"))
            pav = es.enter_context(tc.tile_pool(name="pav", bufs=1, space="PSUM"))
            pwork = es.enter_context(tc.tile_pool(name="pwork", bufs=2, space="PSUM"))

            # ---- constants
            def wtile(dram, shape, dt, name):
                t = constp.tile(shape, dt, tag=name, name=name)
                nc.sync.dma_start(out=t[:], in_=dram[:])
                return t

            wq_h = wtile(wqh, [128, 2, 768], fp8, "wqh")
            wq_l = wtile(wql, [128, 2, 768], fp8, "wql")
            wk_h = wtile(wkh, [128, 2, 768], fp8, "wkh")
            wk_l = wtile(wkl, [128, 2, 768], fp8, "wkl")
            wv_h = wtile(wvh, [128, 2, 512], fp8, "wvh")
            wv_l = wtile(wvl, [128, 2, 512], fp8, "wvl")
            wo_t = wtile(woh, [128, 4, 256], bf16, "woh")
            bo_r = wtile(bo, [1, 256], f32r, "bor")
            onerow = wtile(onesd, [1, 512], f32r, "onerow")
            ones_e = constp.tile([128, 2, 128], fp8, tag="onese", name="ones_e")
            nc.gpsimd.memset(ones_e[:], 0.0)
            nc.gpsimd.memset(ones_e[:, :, 0:64], 1.0)
            ones_o = constp.tile([128, 2, 128], fp8, tag="oneso", name="ones_o")
            nc.gpsimd.memset(ones_o[:], 0.0)
            nc.gpsimd.memset(ones_o[:, :, 64:128], 1.0)

            # ---- per-stripe input tiles
            x_t, xl_t, sk_t, skl_t, cos_t, sin_t = {}, {}, {}, {}, {}, {}
            for s in (0, 1):
                x_t[s] = inp.tile([128, 2, SPOS], fp8, tag=f"x{s}", name=f"x{s}")
                xl_t[s] = inp.tile([128, 2, SPOS], fp8, tag=f"xl{s}", name=f"xl{s}")
                sk_t[s] = inp.tile([128, 2, SPOS], fp8, tag=f"sk{s}", name=f"sk{s}")
                skl_t[s] = inp.tile([128, 2, SPOS], fp8, tag=f"skl{s}", name=f"skl{s}")
                cos_t[s] = inp.tile([128, SPOS], bf16, tag=f"cos{s}", name=f"cos{s}")
                sin_t[s] = inp.tile([128, SPOS], bf16, tag=f"sin{s}", name=f"sin{s}")

            def dma_inputs(s):
                nc.sync.dma_start(out=x_t[s][:], in_=xs[s])
                nc.sync.dma_start(out=xl_t[s][:], in_=xss[s])
                nc.sync.dma_start(out=sk_t[s][:], in_=sks[s])
                nc.sync.dma_start(out=skl_t[s][:], in_=skss[s])
                nc.sync.dma_start(out=cos_t[s][:], in_=cos4[s])
                nc.sync.dma_start(out=sin_t[s][:], in_=sin4[s])

            # ---- per-stripe slabs
            q_sl, k_sl, ks_sl, vx_sl, vs_sl, u_sl = {}, {}, {}, {}, {}, {}
            for s in (0, 1):
                for g in (0, 1):
                    q_sl[s, g] = slabp.tile([128, 2, SPOS], fp8, tag=f"q{s}{g}", name=f"q{s}{g}")
                    k_sl[s, g] = slabp.tile([128, 2, SPOS], fp8, tag=f"k{s}{g}", name=f"k{s}{g}")
                    ks_sl[s, g] = slabp.tile([128, 2, SPOS], fp8, tag=f"ks{s}{g}", name=f"ks{s}{g}")
                vx_sl[s] = slabp.tile([128, 16, 768], fp8, tag=f"vx{s}", name=f"vx{s}")
                vs_sl[s] = slabp.tile([128, 16, 768], fp8, tag=f"vs{s}", name=f"vs{s}")
                for vt in (vx_sl[s], vs_sl[s]):
                    zv = vt[:].rearrange("p c (pr t x) -> p (c pr t) x", pr=4, t=3, x=64)
                    nc.gpsimd.memset(zv[:, 1::3, :], 0.0)
                u_sl[s] = slabp.tile([128, 4, SPOS], bf16, tag=f"u{s}", name=f"u{s}")

            # chunk column offsets in the 768-col weight tiles
            PM = {0: 0, 1: 128}
            PASS = {0: 256, 1: 384}
            ROT = {0: 512, 1: 640}

            def proj_units(s):
                """Generator: projections of stripe s in small tile units."""
                for nt in range(NT):
                    ntsl = slice(nt * 512, (nt + 1) * 512)
                    for pi, (wh, wl, rhs_h, rhs_l, dsts) in enumerate((
                        (wq_h, wq_l, x_t[s], xl_t[s], q_sl),
                        (wk_h, wk_l, x_t[s], xl_t[s], k_sl),
                        (wk_h, wk_l, sk_t[s], skl_t[s], ks_sl),
                    )):
                        for g in (0, 1):
                            pm = pwork.tile([128, 512], f32, tag="pw", name="pm")
                            nc.tensor.matmul(pm[:], wh[:, :, PM[g]:PM[g] + 128],
                                             rhs_h[:, :, ntsl], start=True, stop=False,
                                             perf_mode=DR)
                            nc.tensor.matmul(pm[:], wl[:, :, PM[g]:PM[g] + 128],
                                             rhs_l[:, :, ntsl], start=False, stop=True,
                                             perf_mode=DR)
                            pr = pwork.tile([128, 512], f32, tag="pw", name="pr")
                            nc.tensor.matmul(pr[:], wh[:, :, ROT[g]:ROT[g] + 128],
                                             rhs_h[:, :, ntsl], start=True, stop=False,
                                             perf_mode=DR)
                            nc.tensor.matmul(pr[:], wl[:, :, ROT[g]:ROT[g] + 128],
                                             rhs_l[:, :, ntsl], start=False, stop=True,
                                             perf_mode=DR)
                            t1 = tmpp.tile([128, 512], bf16, tag="t1", name="t1")
                            nc.vector.tensor_tensor(out=t1[:], in0=pm[:],
                                                    in1=cos_t[s][:, ntsl], op=MUL)
                            t2 = tmpp.tile([128, 512], bf16, tag="t2", name="t2")
                            nc.vector.tensor_tensor(out=t2[:], in0=pr[:],
                                                    in1=sin_t[s][:, ntsl], op=MUL)
                            nc.gpsimd.tensor_tensor(out=dsts[s, g][:, 0, ntsl],
                                                    in0=t1[:], in1=t2[:], op=ADD)
                            yield
                            pp = pwork.tile([128, 512], f32, tag="pw", name="pp")
                            nc.tensor.matmul(pp[:], wh[:, :, PASS[g]:PASS[g] + 128],
                                             rhs_h[:, :, ntsl], start=True, stop=False,
                                             perf_mode=DR)
                            nc.tensor.matmul(pp[:], wl[:, :, PASS[g]:PASS[g] + 128],
                                             rhs_l[:, :, ntsl], start=False, stop=True,
                                             perf_mode=DR)
                            # pass-through eviction: split DVE / ACT by parity
                            if (pi + g) % 2 == 0:
                                nc.vector.tensor_copy(dsts[s, g][:, 1, ntsl], pp[:])
                            else:
                                nc.scalar.copy(dsts[s, g][:, 1, ntsl], pp[:])
                            yield
                    # V projections (transposed orientation), 4 pos-chunks/nt/half
                    for hi, (src_h, src_l, vdst) in enumerate(((x_t[s], xl_t[s], vx_sl[s]),
                                                               (sk_t[s], skl_t[s], vs_sl[s]))):
                        for pc in range(4):
                            cidx = nt * 4 + pc
                            psl = slice(nt * 512 + pc * 128, nt * 512 + pc * 128 + 128)
                            pv = pwork.tile([128, 512], f32, tag="pw", name="pv")
                            nc.tensor.matmul(pv[:], src_h[:, :, psl], wv_h[:],
                                             start=True, stop=False, perf_mode=DR)
                            nc.tensor.matmul(pv[:], src_l[:, :, psl], wv_l[:],
                                             start=False, stop=True, perf_mode=DR)
                            vo = vdst[:, cidx, :].rearrange(
                                "p (pr t x) -> p pr t x", pr=4, t=3, x=64)[:, :, ::2, :]
                            vi = pv[:].rearrange("p (pr t x) -> p pr t x",
                                                 pr=4, t=2, x=64)
                            nc.scalar.copy(vo, vi)
                            yield

            def outproj_units(s):
                """Output projection of stripe s; bias seeded via K=1 matmul,
                result DMA'd straight from PSUM."""
                for nt in range(NT):
                    ntsl = slice(nt * 512, (nt + 1) * 512)
                    for m in (0, 1):
                        fp = pwork.tile([128, 512], f32, tag="pw", name="fp")
                        nc.tensor.matmul(fp[:], bo_r[:, m * 128:(m + 1) * 128],
                                         onerow[:], start=True, stop=False)
                        for kt in range(4):
                            nc.tensor.matmul(fp[:], wo_t[:, kt, m * 128:(m + 1) * 128],
                                             u_sl[s][:, kt, ntsl],
                                             start=False, stop=(kt == 3))
                        osb = outp.tile([128, 512], f32, tag="ot", name="osb")
                        nc.scalar.copy(osb[:], fp[:])
                        nc.sync.dma_start(out=out[s, m, :, ntsl], in_=osb[:])
                        yield

            def emit_sim(s, y, j):
                """sim for window pair (y, 2j), (y, 2j+1); returns exp tile."""
                st = psim.tile([128, 8, 256], f32, tag="sim", name="sim")
                for o, h in ((0, 2 * j), (1, 2 * j + 1)):
                    g, po = h // 4, (h % 4) * 32
                    for mc in range(4):
                        ksl = (k_sl if mc < 2 else ks_sl)[s, g]
                        lhsT = ksl[po:po + 32, :,
                                   y * 256 + (mc % 2) * 128:
                                   y * 256 + (mc % 2) * 128 + 128]
                        rhs = q_sl[s, g][po:po + 32, :, y * 256:y * 256 + 256]
                        nc.tensor.matmul(st[:, 4 * o + mc, :], lhsT, rhs,
                                         start=True, stop=True, perf_mode=DR,
                                         tile_position=(po, 0))
                et = expp.tile([128, 8, 256], fp8, tag="exp", name="exp")
                return st, et

            def emit_exp(st, et):
                nc.scalar.activation(et[:], st[:], AF.Exp, scale=float(DH) ** -0.5)

            def emit_av_batch(s, y, jb, et0, et1):
                """AV + denominator for pairs (y, jb), (y, jb+1); one recip +
                one mult normalize both. pav layout: [av0 av1 d0 d1] x 256."""
                at = pav.tile([128, 1024], f32, tag="av", name="av")
                for bi, (j, et) in enumerate(((jb, et0), (jb + 1, et1))):
                    # windows (y, 2j) -> rows 0:64 and (y, 2j+1) -> rows 64:128
                    # via zero-padded lhsT; 4 matmuls accumulate one region.
                    step = 0
                    for o in (0, 1):
                        pb = j * 192 + o * 64
                        for mc, vsl in ((0, vx_sl[s]), (1, vs_sl[s])):
                            nc.tensor.matmul(at[:, bi * 256:bi * 256 + 256],
                                             vsl[:, 2 * y:2 * y + 2, pb:pb + 128],
                                             et[:, 4 * o + 2 * mc:4 * o + 2 * mc + 2, :],
                                             start=(step == 0), stop=(step == 3),
                                             perf_mode=DR)
                            step += 1
                    step = 0
                    for o, ot in ((0, ones_e), (1, ones_o)):
                        for mc in (0, 1):
                            nc.tensor.matmul(at[:, 512 + bi * 256:512 + bi * 256 + 256],
                                             ot[:],
                                             et[:, 4 * o + 2 * mc:4 * o + 2 * mc + 2, :],
                                             start=(step == 0), stop=(step == 3),
                                             perf_mode=DR)
                            step += 1
                rt = rtp.tile([128, 512], f32, tag="rt", name="rt")
                with nc.allow_low_precision(reason="softmax denominator reciprocal"):
                    nc.vector.reciprocal(rt[:], at[:, 512:1024])
                nc.vector.tensor_tensor(
                    out=u_sl[s][:, jb:jb + 2, y * 256:y * 256 + 256],
                    in0=at[:, 0:512], in1=rt[:], op=MUL)

            def attn_phase(s, filler, pulls, on_y_done=None):
                prev = None
                def flush_prev():
                    nonlocal prev
                    if prev is not None:
                        emit_av_batch(s, *prev)
                        if prev[1] == 2 and on_y_done is not None:
                            on_y_done(prev[0])
                        prev = None
                for y in range(YW):
                    for jb in (0, 2):
                        st0, et0 = emit_sim(s, y, jb)
                        emit_exp(st0, et0)
                        st1, et1 = emit_sim(s, y, jb + 1)
                        emit_exp(st1, et1)
                        flush_prev()
                        prev = (y, jb, et0, et1)
                        for _ in range(2 * pulls):
                            next(filler, None)
                flush_prev()
                for _ in range(4):
                    next(filler, None)

            def drain(gen):
                for _ in gen:
                    pass

            # ---- pipeline
            dma_inputs(0)
            dma_inputs(1)
            drain(proj_units(0))
            attn_phase(0, proj_units(1), pulls=3)
            attn_phase(1, outproj_units(0), pulls=1)
            drain(outproj_units(1))

    _split_excess_waits(nc)
    return nc


def _split_excess_waits(nc, max_waits=1):
    """walrus accepts one sync-wait command per instruction; hoist excess
    waits onto same-engine NoOps inserted just before."""
    import bass_rust
    import concourse.mybir as mybir
    n_added = 0
    for f in nc.m.functions:
        for bb in f.blocks:
            insts = list(bb.instructions)
            new = []
            dirty = False
            for inst in insts:
                si = inst.sync_info
                if si is not None and len(si.on_wait) > max_waits:
                    waits = list(si.on_wait)
                    for wt in waits[:-max_waits]:
                        nop = mybir.InstNoOp(name=f"{inst.name}-ws{n_added}",
                                             ins=[], outs=[])
                        nop.engine = inst.engine
                        nop.sync_info = bass_rust.SyncInfo(on_wait=[wt], on_update=[])
                        new.append(nop)
                        n_added += 1
                    inst.sync_info = bass_rust.SyncInfo(
                        on_wait=waits[-max_waits:], on_update=list(si.on_update))
                    dirty = True
                new.append(inst)
            if dirty:
                bb.instructions = new
    return n_added


def _window_major(a):
    """[..., 16, 128] spatial block -> [..., 2048] window-major positions."""
    lead = a.shape[:-2]
    return (a.reshape(*lead, WIN, YW, WIN)
             .swapaxes(-3, -2)
             .reshape(*lead, SPOS))


def _pack_kt(a):
    """[256, N] -> [128, 2, N] (channel c = kt*128 + p)."""
    n = a.shape[-1]
    return np.ascontiguousarray(a.reshape(2, 128, n).transpose(1, 0, 2))


def _rot_weights(Wm):
    R = np.zeros_like(Wm)
    for h in range(HEADS):
        b0 = h * DH
        for i in range(DRR // 2):
            R[b0 + 2 * i] = -Wm[b0 + 2 * i + 1]
            R[b0 + 2 * i + 1] = Wm[b0 + 2 * i]
    return R


def _qk_weight(Wm):
    """[512, 256] -> [768, 256] rows ordered (pm-g0, pm-g1, pass-g0, pass-g1,
    rot-g0, rot-g1), each chunk of 128 = 4 heads x 32 channels."""
    R = _rot_weights(Wm)
    rows = np.empty((768, C), np.float32)
    for g in (0, 1):
        for p in range(128):
            h, d = 4 * g + p // 32, p % 32
            rows[g * 128 + p] = Wm[h * DH + d]              # pm (rope dims)
            rows[256 + g * 128 + p] = Wm[h * DH + 32 + d]   # pass
            rows[512 + g * 128 + p] = R[h * DH + d]         # rot
    return rows


def _make_core_inputs(x, skip, time_emb, sin, cos, Wq, Wkv, Wout, bout):
    import ml_dtypes
    f8 = ml_dtypes.float8_e4m3
    bfd = ml_dtypes.bfloat16

    def split_resid(rows):  # [M, 256] f32 -> hi/lo [128, 2, M] fp8
        hi = np.asarray(rows, f8).astype(np.float32)
        lo = SC * (np.asarray(rows, np.float32) - hi)
        return (np.asarray(_pack_kt(hi.T), f8),
                np.asarray(_pack_kt(np.asarray(lo, f8).astype(np.float32).T), f8))

    Wk_, Wv_ = Wkv[:512], Wkv[512:]
    wqh_a, wql_a = split_resid(_qk_weight(Wq))
    wkh_a, wkl_a = split_resid(_qk_weight(Wk_))
    wvh_a, wvl_a = split_resid(Wv_)      # natural col order h*64+d

    # wo rows: u-channel (p, kt): h = 2*kt + p//64, d = p%64
    wo_rows = np.empty((128, 4, C), np.float32)
    for p in range(128):
        for kt in range(4):
            h, d = 2 * kt + p // 64, p % 64
            wo_rows[p, kt] = Wout[:, h * DH + d]
    woh_a = np.asarray(wo_rows, bfd)
    bo_a = bout.reshape(1, C).astype(np.float32).copy()

    in_maps = []
    for core in range(NCORES):
        b = core // 4
        xbs = [2 * (core % 4), 2 * (core % 4) + 1]
        xs_c = np.empty((2, 128, 2, SPOS), f8)
        xss_c = np.empty((2, 128, 2, SPOS), f8)
        sk_c = np.empty((2, 128, 2, SPOS), f8)
        skss_c = np.empty((2, 128, 2, SPOS), f8)
        cos_c = np.empty((2, 128, SPOS), bfd)
        sin_c = np.empty((2, 128, SPOS), bfd)
        for si, xb in enumerate(xbs):
            rs = slice(xb * WIN, (xb + 1) * WIN)
            xf = _window_major((x[b, :, rs, :] + time_emb[b][:, None, None])
                               .astype(np.float32))
            skf = _window_major(skip[b, :, rs, :].astype(np.float32))
            xs_c[si] = np.asarray(_pack_kt(xf), f8)
            xss_c[si] = np.asarray(_pack_kt(xf / SC), f8)
            sk_c[si] = np.asarray(_pack_kt(skf), f8)
            skss_c[si] = np.asarray(_pack_kt(skf / SC), f8)
            cw = _window_major(cos[rs].transpose(2, 0, 1))   # [32, 2048]
            sw = _window_major(sin[rs].transpose(2, 0, 1))
            cos_c[si] = np.asarray(np.tile(cw, (4, 1)), bfd)
            sin_c[si] = np.asarray(np.tile(sw, (4, 1)), bfd)
        in_maps.append({
            "xs": xs_c, "xss": xss_c, "sks": sk_c, "skss": skss_c,
            "cos4": cos_c, "sin4": sin_c,
            "wqh": wqh_a, "wql": wql_a, "wkh": wkh_a, "wkl": wkl_a,
            "wvh": wvh_a, "wvl": wvl_a, "woh": woh_a, "bo": bo_a,
            "onesd": np.ones((1, 512), np.float32),
        })
    return in_maps


def _assemble(results):
    out_full = np.empty((B, C, H, W), np.float32)
    for core in range(NCORES):
        b = core // 4
        xbs = [2 * (core % 4), 2 * (core % 4) + 1]
        o = results[core]["out"]          # [2, 2, 128, 2048] window-major
        for si, xb in enumerate(xbs):
            ch = o[si].reshape(C, YW, WIN, WIN)
            blk = ch.swapaxes(1, 2).reshape(C, WIN, W)
            out_full[b, :, xb * WIN:(xb + 1) * WIN, :] = blk
    return out_full


def get_nc():
    if "nc" not in _CACHE:
        _CACHE["nc"] = _build()
    return _CACHE["nc"]


def kernel(x, skip, time_emb, sin, cos, Wq, Wkv, Wout, bout):
    from concourse.bass_utils import run_bass_kernel_spmd
    args = [np.asarray(a, dtype=np.float32) for a in
            (x, skip, time_emb, sin, cos, Wq, Wkv, Wout, bout)]
    nc = get_nc()
    in_maps = _make_core_inputs(*args)
    res = run_bass_kernel_spmd(nc, in_maps, list(range(NCORES)), trace=False)
    return _assemble(res.results)
